# revision 4
# baseline (speedup 1.0000x reference)
import sys
sys.path.insert(0, "/opt/trn_rl_repo")
import numpy as np
import ml_dtypes

import concourse.bass as bass
import concourse.mybir as mybir
from concourse import bacc
from concourse.tile import TileContext
from concourse.bass_utils import run_bass_kernel_spmd

F32 = mybir.dt.float32
BF16 = mybir.dt.bfloat16
I32 = mybir.dt.int32
I16 = mybir.dt.int16
P = 128
W = 8               # cores
NEG = 0.2
EPS = 1e-16
C1 = 4.0            # exp shift layer 1 (softmax is shift-invariant)
C2 = 2.0            # exp shift layer 2
SUB = 64            # dst sub-block width (psum strip size); strips at 0 / 64
BF = ml_dtypes.bfloat16

AL = mybir.AluOpType
AF = mybir.ActivationFunctionType

_prog_cache = {}


def _wrap16(idx_flat):
    # flat j-ordered int array [S*128] -> dma_gather idx layout [128, S*8] i16
    wr = idx_flat.reshape(-1, 16).T.astype(np.int16)   # [16, S*8]
    return np.tile(wr, (8, 1)).copy()


def _host_prep(x, edge_index, W1, a1s, a1d, b1, W2, a2s, a2d, b2):
    N, IND = x.shape
    E = edge_index.shape[1]
    H, HID = a1s.shape
    OD = a2s.shape[1]
    D1 = H * HID
    NS = N // W
    NSP = ((NS + 127) // 128) * 128
    NT1 = ((N + 255) // 256) * 256
    NT2 = W * NSP

    src = np.asarray(edge_index[0]).astype(np.int64)
    dst = np.asarray(edge_index[1]).astype(np.int64)

    w1s = np.stack([W1[:, h * HID:(h + 1) * HID] @ a1s[h] for h in range(H)], 1)
    w1d = np.stack([W1[:, h * HID:(h + 1) * HID] @ a1d[h] for h in range(H)], 1)
    W1ext = np.concatenate([W1, w1s, w1d], axis=1)                  # [256,144]
    W2ext = np.concatenate(
        [W2, W2 @ a2s[0][:, None], W2 @ a2d[0][:, None]], 1)        # [128,66]

    spc = NSP // SUB                       # sub-blocks per core (98)
    order = np.argsort(dst, kind="stable")
    src_s, dst_s = src[order], dst[order]
    ecore = dst_s // NS
    dloc = dst_s - ecore * NS
    sub_all = ecore * spc + dloc // SUB
    counts = np.bincount(sub_all, minlength=W * spc)
    T_sub = int(np.ceil(counts.max() / 128))
    S = spc * T_sub
    NBLK = spc // 2

    srcA = np.zeros((W, S * 128), np.int64)
    dofA = np.full((W, S * 128), 999.0, np.float32)
    dlocA = np.zeros((W, S * 128), np.int64)

    starts = np.zeros(W * spc + 1, np.int64)
    np.cumsum(counts, out=starts[1:])
    ar = np.arange(S * 128)
    for c in range(W):
        for sb in range(spc):
            g = c * spc + sb
            n = counts[g]
            if n == 0:
                continue
            e0 = starts[g]
            pos = sb * T_sub * 128 + np.arange(n)
            srcA[c, pos] = src_s[e0:e0 + n]
            dofA[c, pos] = (dloc[e0:e0 + n] % SUB).astype(np.float32)
            dlocA[c, pos] = dloc[e0:e0 + n]

    bA = (srcA & 1).astype(np.float32)
    scoreA = srcA // NS
    src2A = scoreA * NSP + (srcA - scoreA * NS)
    b2A = (src2A & 1).astype(np.float32)

    xT = np.zeros((256, NT1), BF)
    xT[:, :N] = np.asarray(x, np.float32).T.astype(BF)

    per_core = []
    for c in range(W):
        # lane-major [128, S] views: edge j -> (lane j%128, slot j//128)
        def lm(a, dt):
            return np.ascontiguousarray(
                a.reshape(S, 128).T.astype(dt))
        per_core.append({
            "xT": xT,
            "sh_xT": np.ascontiguousarray(xT[:, c * NS: c * NS + NSP]),
            "w1e": W1ext.astype(BF),
            "w2e": W2ext.astype(BF),
            "sidx": _wrap16(srcA[c] >> 1),
            "didx": _wrap16(dlocA[c]),
            "s2idx": _wrap16(src2A[c] >> 1),
            "dof": lm(dofA[c], np.float32),
            "bpar": lm(bA[c], np.float32),
            "bnpar": lm(1.0 - bA[c], np.float32),
            "b2par": lm(b2A[c], np.float32),
            "b2npar": lm(1.0 - b2A[c], np.float32),
            "b1r": np.asarray(b1, np.float32).reshape(1, D1),
            "b2r": np.asarray(b2, np.float32).reshape(1, OD),
        })
    dims = dict(N=N, E=E, H=H, HID=HID, OD=OD, D1=D1, NS=NS, NSP=NSP,
                NT1=NT1, NT2=NT2, T_sub=T_sub, S=S, NBLK=NBLK, spc=spc)
    return per_core, dims


def _build_program(d):
    NT1, NT2, NSP = d["NT1"], d["NT2"], d["NSP"]
    T_sub, S, NBLK = d["T_sub"], d["S"], d["NBLK"]
    H, OD, D1 = d["H"], d["OD"], d["D1"]
    TB = 2 * T_sub
    NI = TB * 128
    NTIL = NT1 // 128
    SUPER = 32
    NST = NSP // 128          # 49 shard tiles

    nc = bacc.Bacc()
    xT_d = nc.dram_tensor("xT", [256, NT1], BF16, kind="ExternalInput")
    shxT_d = nc.dram_tensor("sh_xT", [256, NSP], BF16, kind="ExternalInput")
    w1e_d = nc.dram_tensor("w1e", [256, 144], BF16, kind="ExternalInput")
    w2e_d = nc.dram_tensor("w2e", [128, 66], BF16, kind="ExternalInput")
    sidx_d = nc.dram_tensor("sidx", [128, S * 8], I16, kind="ExternalInput")
    didx_d = nc.dram_tensor("didx", [128, S * 8], I16, kind="ExternalInput")
    s2idx_d = nc.dram_tensor("s2idx", [128, S * 8], I16, kind="ExternalInput")
    dof_d = nc.dram_tensor("dof", [128, S], F32, kind="ExternalInput")
    bpar_d = nc.dram_tensor("bpar", [128, S], F32, kind="ExternalInput")
    bnpar_d = nc.dram_tensor("bnpar", [128, S], F32, kind="ExternalInput")
    b2par_d = nc.dram_tensor("b2par", [128, S], F32, kind="ExternalInput")
    b2npar_d = nc.dram_tensor("b2npar", [128, S], F32, kind="ExternalInput")
    b1r_d = nc.dram_tensor("b1r", [1, D1], F32, kind="ExternalInput")
    b2r_d = nc.dram_tensor("b2r", [1, OD], F32, kind="ExternalInput")
    out_d = nc.dram_tensor("out", [NSP, OD], F32, kind="ExternalOutput")

    T1 = nc.dram_tensor("T1", [NT1 // 2, 384], BF16, kind="Internal")
    Td1 = nc.dram_tensor("Td1", [NSP, 128], BF16, kind="Internal")
    ag_in = nc.dram_tensor("ag_in", [NSP, 128], BF16, kind="Internal")
    ag_out = nc.dram_tensor("ag_out", [NT2, 128], BF16, kind="Internal",
                            addr_space="Shared")
    T1v = T1.rearrange("q (two c) -> (q two) c", two=2)      # [NT1, 192]
    T2p = ag_out.rearrange("(q two) c -> q (two c)", two=2)  # [NT2//2, 256]

    def bin_(ap, n):    # broadcast new innermost dim
        return ap.to_broadcast(list(ap.shape) + [n])

    def bmid(ap, n):    # [p, ...] -> [p, n, ...]
        a = ap.ap
        return bass.AP(ap.tensor, ap.offset, [a[0], [0, n]] + list(a[1:]))

    with TileContext(nc) as tc:
        cp = tc.alloc_tile_pool(name="const", bufs=1)
        iota_b = cp.tile([P, SUB], BF16)
        ii = cp.tile([P, SUB], I32)
        nc.gpsimd.iota(ii[:], pattern=[[1, SUB]], base=0, channel_multiplier=0)
        nc.vector.tensor_copy(iota_b[:], ii[:])
        negC1 = cp.tile([P, 1], F32)
        nc.vector.memset(negC1[:], -C1)
        negC2 = cp.tile([P, 1], F32)
        nc.vector.memset(negC2[:], -C2)
        b1r = cp.tile([P, D1], F32)
        nc.sync.dma_start(b1r[:], bass.AP(b1r_d, 0, [[0, P], [1, D1]]))
        b2r = cp.tile([P, OD], F32)
        nc.sync.dma_start(b2r[:], bass.AP(b2r_d, 0, [[0, P], [1, OD]]))
        # identity (for PE transpose): ident[p,q] = (q - p == 0)
        iota128 = cp.tile([P, P], BF16)
        i128 = cp.tile([P, P], I32)
        nc.gpsimd.iota(i128[:], pattern=[[1, P]], base=0, channel_multiplier=-1)
        nc.vector.tensor_copy(iota128[:], i128[:])
        zrow = cp.tile([P, P], BF16)
        nc.vector.memset(zrow[:], 0.0)
        ident = cp.tile([P, P], BF16)
        nc.vector.tensor_tensor(out=ident[:], in0=iota128[:], in1=zrow[:],
                                op=AL.is_equal)
        # persistent state
        dofb = cp.tile([P, S], BF16)
        doff32 = cp.tile([P, S], F32)
        nc.sync.dma_start(doff32[:], dof_d[:])
        nc.vector.tensor_copy(dofb[:], doff32[:])
        bp = cp.tile([P, S], BF16)
        nc.gpsimd.dma_start(bp[:], bpar_d[:])
        bn = cp.tile([P, S], BF16)
        nc.gpsimd.dma_start(bn[:], bnpar_d[:])
        b2p = cp.tile([P, S], BF16)
        nc.gpsimd.dma_start(b2p[:], b2par_d[:])
        b2n = cp.tile([P, S], BF16)
        nc.gpsimd.dma_start(b2n[:], b2npar_d[:])
        out1 = cp.tile([P, NST, D1], F32)
        out2 = cp.tile([P, NST, OD], F32)
        w2sb = cp.tile([P, 66], BF16)
        nc.sync.dma_start(w2sb[:], w2e_d[:])

        # ===== phase 1: T1 rows = [x@W1 | alpha_src | alpha_dst | pad] =====
        with tc.tile_pool(name="ph1", bufs=2) as p1, \
             tc.tile_pool(name="ph1w", bufs=1) as p1w, \
             tc.tile_pool(name="ph1ps", bufs=4, space="PSUM") as p1ps:
            w1sb = p1w.tile([P, 2, 144], BF16)
            nc.sync.dma_start(w1sb[:], w1e_d.rearrange("(k p) c -> p k c", p=P))
            nst = (NTIL + SUPER - 1) // SUPER
            for st in range(nst):
                ntile = min(SUPER, NTIL - st * SUPER)
                nn = ntile * 128
                n0 = st * SUPER * 128
                xsl = p1.tile([P, 2, SUPER * 128], BF16, tag="xsl")
                for k in range(2):
                    nc.sync.dma_start(xsl[:, k, 0:nn],
                                      xT_d[k * 128:(k + 1) * 128, n0:n0 + nn])
                tb = p1.tile([P, SUPER, 144], BF16, tag="tb")
                for j in range(ntile):
                    ps = p1ps.tile([P, 144], F32, space="PSUM", tag="pps")
                    for k in range(2):
                        nc.tensor.matmul(
                            out=ps[:], lhsT=xsl[:, k, j * 128:(j + 1) * 128],
                            rhs=w1sb[:, k, :], start=(k == 0), stop=(k == 1))
                    nc.scalar.copy(tb[:, j, :], ps[:])
                nc.sync.dma_start(
                    T1v[n0:n0 + nn, 0:144].rearrange("(t p) c -> p t c", p=P),
                    tb[:, 0:ntile, :])

            # Td1 rows = [alpha_dst(local shard) | pad] from sh_xT @ w1d
            xs2 = p1w.tile([P, 2, NSP], BF16)
            nc.sync.dma_start(xs2[:], shxT_d.rearrange("(k p) n -> p k n", p=P))
            tdd = p1.tile([P, NST, 8], BF16, tag="tdd")
            for j in range(NST):
                ps = p1ps.tile([P, 8], F32, space="PSUM", tag="ppd")
                for k in range(2):
                    nc.tensor.matmul(
                        out=ps[:], lhsT=xs2[:, k, j * 128:(j + 1) * 128],
                        rhs=w1sb[:, k, 136:144], start=(k == 0), stop=(k == 1))
                nc.scalar.copy(tdd[:, j, :], ps[:])
            nc.sync.dma_start(
                Td1[:, 0:8].rearrange("(t p) c -> p t c", p=P), tdd[:])

        # ===== layer 1 edge phase =====
        with tc.tile_pool(name="l1", bufs=2) as lp, \
             tc.tile_pool(name="l1s", bufs=3) as ls, \
             tc.tile_pool(name="l1ps", bufs=2, space="PSUM") as lps:
            for blk in range(NBLK):
                j0 = blk * TB * 8
                c0 = blk * TB
                si = ls.tile([P, TB * 8], I16, tag="si")
                nc.sync.dma_start(si[:], sidx_d[:, j0:j0 + TB * 8])
                di = ls.tile([P, TB * 8], I16, tag="di")
                nc.sync.dma_start(di[:], didx_d[:, j0:j0 + TB * 8])
                Gp = lp.tile([P, TB, 384], BF16, tag="Gp")
                for i0 in range(0, TB, 4):
                    w4 = min(4, TB - i0)
                    nc.gpsimd.dma_gather(
                        out_ap=Gp[:, i0:i0 + w4, :], in_ap=T1[:],
                        idxs_ap=si[:, i0 * 8:(i0 + w4) * 8],
                        num_idxs=w4 * 128, num_idxs_reg=w4 * 128, elem_size=384)
                Ad = lp.tile([P, TB, 128], BF16, tag="Ad")
                for i0 in range(0, TB, 4):
                    w4 = min(4, TB - i0)
                    nc.gpsimd.dma_gather(
                        out_ap=Ad[:, i0:i0 + w4, :], in_ap=Td1[:],
                        idxs_ap=di[:, i0 * 8:(i0 + w4) * 8],
                        num_idxs=w4 * 128, num_idxs_reg=w4 * 128, elem_size=128)
                # alpha_src per edge (pick pair half), logits, ex
                asE = ls.tile([P, TB, H], F32, tag="asE")
                nc.vector.tensor_tensor(out=asE[:], in0=Gp[:, :, 128:136],
                                        in1=bin_(bn[:, c0:c0 + TB], H), op=AL.mult)
                ahi = ls.tile([P, TB, H], F32, tag="ahi")
                nc.vector.tensor_tensor(out=ahi[:], in0=Gp[:, :, 320:328],
                                        in1=bin_(bp[:, c0:c0 + TB], H), op=AL.mult)
                adf = ls.tile([P, TB, H], F32, tag="adf")
                nc.vector.tensor_copy(adf[:], Ad[:, :, 0:H])
                z = ls.tile([P, TB, H], F32, tag="z")
                nc.vector.tensor_tensor(out=z[:], in0=asE[:], in1=ahi[:], op=AL.add)
                nc.vector.tensor_tensor(out=z[:], in0=z[:], in1=adf[:],
                                        op=AL.add)
                nc.vector.scalar_tensor_tensor(out=z[:], in0=z[:], scalar=NEG,
                                               in1=z[:], op0=AL.mult, op1=AL.max)
                ex = ls.tile([P, TB, H], F32, tag="ex")
                nc.scalar.activation(ex[:], z[:], AF.Exp, bias=negC1[:])
                nc.vector.tensor_copy(Gp[:, :, 128:136], ex[:])
                exlo = ls.tile([P, TB, H], BF16, tag="exlo")
                nc.vector.tensor_tensor(out=exlo[:], in0=ex[:],
                                        in1=bin_(bn[:, c0:c0 + TB], H), op=AL.mult)
                exhi = ls.tile([P, TB, H], BF16, tag="exhi")
                nc.vector.tensor_tensor(out=exhi[:], in0=ex[:],
                                        in1=bin_(bp[:, c0:c0 + TB], H), op=AL.mult)
                nc.vector.tensor_tensor(out=Gp[:, :, 0:128], in0=Gp[:, :, 0:128],
                                        in1=bin_(exlo[:], 16), op=AL.mult)
                nc.vector.tensor_tensor(out=Gp[:, :, 192:320],
                                        in0=Gp[:, :, 192:320],
                                        in1=bin_(exhi[:], 16), op=AL.mult)
                nc.vector.memset(Gp[:, :, 320:328], 0.0)
                OH = ls.tile([P, TB, SUB], BF16, tag="OH")
                nc.vector.tensor_tensor(out=OH[:],
                                        in0=bin_(dofb[:, c0:c0 + TB], SUB),
                                        in1=bmid(iota_b[:], TB), op=AL.is_equal)
                ps1 = lps.tile([P, 136], F32, space="PSUM", tag="ps1")
                for t in range(TB):
                    sb, k = divmod(t, T_sub)
                    st = SUB * sb
                    nc.tensor.matmul(out=ps1[st:st + SUB, 0:136],
                                     lhsT=OH[:, t, :], rhs=Gp[:, t, 0:136],
                                     start=(k == 0), stop=False)
                    nc.tensor.matmul(out=ps1[st:st + SUB, 0:136],
                                     lhsT=OH[:, t, :], rhs=Gp[:, t, 192:328],
                                     start=False, stop=(k == T_sub - 1))
                sv = ls.tile([P, H], F32, tag="sv")
                nc.vector.tensor_scalar(out=sv[:], in0=ps1[:, 128:136],
                                        scalar1=EPS, scalar2=None, op0=AL.add)
                inv = ls.tile([P, H], F32, tag="inv")
                nc.vector.reciprocal(inv[:], sv[:])
                nc.vector.tensor_tensor(out=out1[:, blk, :], in0=ps1[:, 0:128],
                                        in1=bin_(inv[:], 16), op=AL.mult)

        # ===== interlayer: ELU, h2' = h2 @ W2ext, allgather =====
        with tc.tile_pool(name="il", bufs=2) as ip, \
             tc.tile_pool(name="ilps", bufs=2, space="PSUM") as ips:
            X = out1[:]
            nc.vector.tensor_tensor(out=X, in0=X, in1=bmid(b1r[:], NST), op=AL.add)
            mn = ip.tile([P, NST, D1], F32, tag="mn")
            nc.vector.tensor_scalar(out=mn[:], in0=X, scalar1=0.0, scalar2=None,
                                    op0=AL.min)
            nc.scalar.activation(mn[:], mn[:], AF.Exp)
            h2 = ip.tile([P, NST, D1], BF16, tag="h2")
            nc.vector.scalar_tensor_tensor(out=h2[:], in0=X, scalar=0.0,
                                           op0=AL.max, in1=mn[:], op1=AL.add)
            nc.vector.tensor_scalar(out=h2[:], in0=h2[:], scalar1=1.0,
                                    scalar2=None, op0=AL.subtract)
            t2 = ip.tile([P, NST, 66], BF16, tag="t2")
            for j in range(NST):
                pT = ips.tile([P, P], BF16, space="PSUM", tag="pT")
                nc.tensor.transpose(out=pT[:], in_=h2[:, j, :], identity=ident[:])
                hT = ip.tile([P, P], BF16, tag="hT")
                nc.vector.tensor_copy(hT[:], pT[:])
                p2 = ips.tile([P, 66], F32, space="PSUM", tag="p2")
                nc.tensor.matmul(out=p2[:], lhsT=hT[:], rhs=w2sb[:],
                                 start=True, stop=True)
                nc.scalar.copy(t2[:, j, :], p2[:])
            nc.sync.dma_start(
                ag_in[:, 0:66].rearrange("(t p) c -> p t c", p=P), t2[:])
            nc.gpsimd.collective_compute(
                "AllGather", AL.bypass, replica_groups=[list(range(W))],
                ins=[ag_in[:]], outs=[ag_out[:]])

        # ===== layer 2 edge phase =====
        with tc.tile_pool(name="l2", bufs=2) as lp2, \
             tc.tile_pool(name="l2s", bufs=3) as ls2, \
             tc.tile_pool(name="l2ps", bufs=2, space="PSUM") as lps2:
            for blk in range(NBLK):
                j0 = blk * TB * 8
                c0 = blk * TB
                si = ls2.tile([P, TB * 8], I16, tag="si2")
                nc.sync.dma_start(si[:], s2idx_d[:, j0:j0 + TB * 8])
                di = ls2.tile([P, TB * 8], I16, tag="di2")
                nc.sync.dma_start(di[:], didx_d[:, j0:j0 + TB * 8])
                Gp = lp2.tile([P, TB, 256], BF16, tag="Gp2")
                for i0 in range(0, TB, 4):
                    w4 = min(4, TB - i0)
                    nc.gpsimd.dma_gather(
                        out_ap=Gp[:, i0:i0 + w4, :], in_ap=T2p[:],
                        idxs_ap=si[:, i0 * 8:(i0 + w4) * 8],
                        num_idxs=w4 * 128, num_idxs_reg=w4 * 128, elem_size=256)
                Ad = lp2.tile([P, TB, 128], BF16, tag="Ad2")
                for i0 in range(0, TB, 4):
                    w4 = min(4, TB - i0)
                    nc.gpsimd.dma_gather(
                        out_ap=Ad[:, i0:i0 + w4, :], in_ap=ag_in[:],
                        idxs_ap=di[:, i0 * 8:(i0 + w4) * 8],
                        num_idxs=w4 * 128, num_idxs_reg=w4 * 128, elem_size=128)
                asE = ls2.tile([P, TB], F32, tag="as2")
                nc.vector.tensor_tensor(out=asE[:], in0=Gp[:, :, 64],
                                        in1=b2n[:, c0:c0 + TB], op=AL.mult)
                ahi = ls2.tile([P, TB], F32, tag="ah2")
                nc.vector.tensor_tensor(out=ahi[:], in0=Gp[:, :, 192],
                                        in1=b2p[:, c0:c0 + TB], op=AL.mult)
                adf = ls2.tile([P, TB], F32, tag="adf2")
                nc.vector.tensor_copy(adf[:], Ad[:, :, 65])
                z = ls2.tile([P, TB], F32, tag="z2")
                nc.vector.tensor_tensor(out=z[:], in0=asE[:], in1=ahi[:], op=AL.add)
                nc.vector.tensor_tensor(out=z[:], in0=z[:], in1=adf[:],
                                        op=AL.add)
                nc.vector.scalar_tensor_tensor(out=z[:], in0=z[:], scalar=NEG,
                                               in1=z[:], op0=AL.mult, op1=AL.max)
                ex = ls2.tile([P, TB], F32, tag="ex2")
                nc.scalar.activation(ex[:], z[:], AF.Exp, bias=negC2[:])
                nc.vector.tensor_copy(Gp[:, :, 64], ex[:])
                exlo = ls2.tile([P, TB], BF16, tag="ex2lo")
                nc.vector.tensor_tensor(out=exlo[:], in0=ex[:],
                                        in1=b2n[:, c0:c0 + TB], op=AL.mult)
                exhi = ls2.tile([P, TB], BF16, tag="ex2hi")
                nc.vector.tensor_tensor(out=exhi[:], in0=ex[:],
                                        in1=b2p[:, c0:c0 + TB], op=AL.mult)
                nc.vector.tensor_tensor(out=Gp[:, :, 0:64], in0=Gp[:, :, 0:64],
                                        in1=bin_(exlo[:], 64), op=AL.mult)
                nc.vector.tensor_tensor(out=Gp[:, :, 128:192],
                                        in0=Gp[:, :, 128:192],
                                        in1=bin_(exhi[:], 64), op=AL.mult)
                nc.vector.memset(Gp[:, :, 192], 0.0)
                OH = ls2.tile([P, TB, SUB], BF16, tag="OH2")
                nc.vector.tensor_tensor(out=OH[:],
                                        in0=bin_(dofb[:, c0:c0 + TB], SUB),
                                        in1=bmid(iota_b[:], TB), op=AL.is_equal)
                ps2 = lps2.tile([P, 65], F32, space="PSUM", tag="ps2")
                for t in range(TB):
                    sb, k = divmod(t, T_sub)
                    st = SUB * sb
                    nc.tensor.matmul(out=ps2[st:st + SUB, 0:65],
                                     lhsT=OH[:, t, :], rhs=Gp[:, t, 0:65],
                                     start=(k == 0), stop=False)
                    nc.tensor.matmul(out=ps2[st:st + SUB, 0:65],
                                     lhsT=OH[:, t, :], rhs=Gp[:, t, 128:193],
                                     start=False, stop=(k == T_sub - 1))
                sv = ls2.tile([P, 1], F32, tag="sv2")
                nc.vector.tensor_scalar(out=sv[:], in0=ps2[:, 64:65],
                                        scalar1=EPS, scalar2=None, op0=AL.add)
                inv = ls2.tile([P, 1], F32, tag="inv2")
                nc.vector.reciprocal(inv[:], sv[:])
                nc.vector.scalar_tensor_tensor(out=out2[:, blk, :],
                                               in0=ps2[:, 0:64], scalar=inv[:],
                                               op0=AL.mult, in1=b2r[:], op1=AL.add)

        nc.sync.dma_start(out_d.rearrange("(t p) c -> p t c", p=P), out2[:])
        cp.release()
    nc.compile()
    return nc


def kernel(**inputs):
    x = np.asarray(inputs["x"], np.float32)
    ei = np.asarray(inputs["edge_index"])
    args = (x, ei, np.asarray(inputs["W1"], np.float32),
            np.asarray(inputs["a1_src"], np.float32),
            np.asarray(inputs["a1_dst"], np.float32),
            np.asarray(inputs["b1"], np.float32),
            np.asarray(inputs["W2"], np.float32),
            np.asarray(inputs["a2_src"], np.float32),
            np.asarray(inputs["a2_dst"], np.float32),
            np.asarray(inputs["b2"], np.float32))
    per_core, dims = _host_prep(*args)
    key = (x.shape, ei.shape, dims["T_sub"])
    if key not in _prog_cache:
        _prog_cache[key] = _build_program(dims)
    nc = _prog_cache[key]
    res = run_bass_kernel_spmd(nc, per_core, core_ids=list(range(W)),
                               trace=False)
    NS = dims["NS"]
    out = np.concatenate([res.results[c]["out"][:NS] for c in range(W)], axis=0)
    return out.astype(np.float32)
